# revision 10
# baseline (speedup 1.0000x reference)
"""Trainium2 Bass kernel for per-head-projection MHA (NaiveMHA).

Problem: B=2, S=2048, D=1024, H=16, DK=64.
  q = einsum('bqd,hdk->bhqk', tgt, Wq); k,v likewise from memory
  attn = softmax(q k^T / sqrt(DK)); out = (attn v concat-heads) @ Wo + bo

Sharding over 8 cores: data-parallel on batch (2) x tensor-parallel on
heads (4 local heads per core).  Each core computes its 4 heads end to
end plus the partial output projection through its slice of Wo's rows;
the host sums the 4 partials per batch and adds bo.

Per-core dataflow (all matmuls fp32r = full-rate 4-byte):
  - stream tgt/memory through PE transposes (identity matmul) to get
    d-major activation chunks
  - qT/kT computed as [dk-pair(128), S] with two heads stacked on the
    partition dim; v computed in natural [S, 4*DK] layout
  - attention in scoresT orientation [sk, sq]: paired K=64 matmuls via
    tile_position row groups, one Exp instruction per head-pair chunk,
    A*V accumulated over sk in PSUM via tile_position col groups,
    softmax row sums via ones-matmul partition reduction, reciprocal,
    K=2 selector-matmul broadcast, normalization applied to the small
    [dk, sq] A*V output (softmax denominator deferred past A*V).
"""

import numpy as np

import concourse.mybir as mybir
import concourse.tile as tile
from concourse import bacc
from concourse.bass_utils import run_bass_kernel_spmd

F32R = mybir.dt.float32r
F32 = mybir.dt.float32
AF = mybir.ActivationFunctionType

B, S, D, H = 2, 2048, 1024, 16
DK = D // H
HL = 4  # local heads per core
NP = 2  # local head pairs
P = 128
DCH = D // P  # 8 d-chunks
QR = 512  # row quarter for streaming transposes
NQR = S // QR  # 4
SQT = 512  # sq tile in attention
NSQT = S // SQT  # 4
NSKC = S // P  # 16 sk chunks
SCALE = 1.0 / float(np.sqrt(DK))

_CACHE = {}


def build_nc(debug=False):
    nc = bacc.Bacc("TRN2", target_bir_lowering=False, debug=False, num_devices=8)

    tgt_d = nc.declare_dram_parameter("tgt", [S, D], F32R, isOutput=False)
    mem_d = nc.declare_dram_parameter("mem", [S, D], F32R, isOutput=False)
    wq_d = nc.declare_dram_parameter("wq", [D, HL * DK], F32R, isOutput=False)
    wk_d = nc.declare_dram_parameter("wk", [D, HL * DK], F32R, isOutput=False)
    wv_d = nc.declare_dram_parameter("wv", [D, HL * DK], F32R, isOutput=False)
    wo_d = nc.declare_dram_parameter("wo", [HL * DK, D], F32R, isOutput=False)
    ident_d = nc.declare_dram_parameter("ident", [P, P], F32R, isOutput=False)
    ones_d = nc.declare_dram_parameter("ones", [P, 1], F32R, isOutput=False)
    sel_d = nc.declare_dram_parameter("sel", [1, 2 * P], F32R, isOutput=False)
    out_d = nc.declare_dram_parameter("out", [S, D], F32R, isOutput=True)
    if debug:
        dbg_qT = nc.declare_dram_parameter("dbg_qT", [P, S // 2], F32R, isOutput=True)
        dbg_kT = nc.declare_dram_parameter("dbg_kT", [P, S // 2], F32R, isOutput=True)
        dbg_v = nc.declare_dram_parameter("dbg_v", [P, HL * DK], F32R, isOutput=True)
        dbg_rec = nc.declare_dram_parameter("dbg_rec", [1, 2 * SQT], F32R, isOutput=True)
        dbg_outT = nc.declare_dram_parameter("dbg_outT", [P, SQT], F32R, isOutput=True)
        dbg_attn = nc.declare_dram_parameter("dbg_attn", [P, 2 * SQT], F32R, isOutput=True)

    with tile.TileContext(nc) as tc:
        with (
            tc.tile_pool(name="const", bufs=1) as const,
            tc.tile_pool(name="w", bufs=1) as wpool,
            tc.tile_pool(name="nat", bufs=6) as nat_pool,
            tc.tile_pool(name="xt", bufs=10) as xt_pool,
            tc.tile_pool(name="qk", bufs=8) as qk_pool,
            tc.tile_pool(name="vp", bufs=16) as v_pool,
            tc.tile_pool(name="attn", bufs=4) as attn_pool,
            tc.tile_pool(name="acc", bufs=4) as acc_pool,
            tc.tile_pool(name="rec", bufs=2) as rec_pool,
            tc.tile_pool(name="outt", bufs=8) as outt_pool,
            tc.tile_pool(name="stage", bufs=3) as stage_pool,
            tc.tile_pool(name="psb", bufs=2, space="PSUM") as ps_big,
            tc.tile_pool(name="psm", bufs=3, space="PSUM") as ps_mid,
        ):
            # constants
            ident = const.tile([P, P], F32R, tag="ident")
            nc.sync.dma_start(out=ident[:], in_=ident_d[:])
            ones = const.tile([P, 1], F32R, tag="ones")
            nc.sync.dma_start(out=ones[:], in_=ones_d[:])
            sel = const.tile([1, 2 * P], F32R, tag="sel")
            nc.sync.dma_start(out=sel[:], in_=sel_d[:])
            # weights
            wq_sb = wpool.tile([P, DCH, HL * DK], F32R, tag="wq")
            nc.sync.dma_start(out=wq_sb[:], in_=wq_d.ap().rearrange("(c p) n -> p c n", p=P))
            wk_sb = wpool.tile([P, DCH, HL * DK], F32R, tag="wk")
            nc.sync.dma_start(out=wk_sb[:], in_=wk_d.ap().rearrange("(c p) n -> p c n", p=P))
            wv_sb = wpool.tile([P, DCH, HL * DK], F32R, tag="wv")
            nc.sync.dma_start(out=wv_sb[:], in_=wv_d.ap().rearrange("(c p) n -> p c n", p=P))
            wo_sb = wpool.tile([P, NP, D], F32R, tag="wo")
            nc.sync.dma_start(out=wo_sb[:], in_=wo_d.ap().rearrange("(c p) n -> p c n", p=P))

            # persistent intermediate tiles
            kT = [[qk_pool.tile([P, S // 2], F32R, tag="qk", name=f"kT_{p}_{h}")
                   for h in range(2)] for p in range(NP)]
            qT = [[qk_pool.tile([P, S // 2], F32R, tag="qk", name=f"qT_{p}_{h}")
                   for h in range(2)] for p in range(NP)]
            v_tiles = [v_pool.tile([P, HL * DK], F32R, tag="v", name=f"v_{i}")
                       for i in range(S // P)]
            outT = [[outt_pool.tile([P, SQT], F32R, tag="outt", name=f"outT_{p}_{t}")
                     for t in range(NSQT)] for p in range(NP)]

            def stream_quarter(src_dram, qr, qk_jobs, with_v):
                """Load rows [qr*QR, (qr+1)*QR) of src, transpose to d-major
                chunks, then run the requested projections at this quarter."""
                nat = []
                for t in range(QR // P):
                    ntile = nat_pool.tile([P, D], F32R, tag="nat")
                    r0 = qr * QR + t * P
                    nc.sync.dma_start(out=ntile[:], in_=src_dram[r0:r0 + P, :])
                    nat.append(ntile)
                xt = []
                for j in range(DCH):
                    xtile = xt_pool.tile([P, QR], F32R, tag="xt")
                    tp = ps_mid.tile([P, QR], F32R, tag="psm")
                    for t in range(QR // P):
                        nc.tensor.transpose(
                            tp[:, t * P:(t + 1) * P], nat[t][:, j * P:(j + 1) * P], ident[:])
                    nc.any.tensor_copy(xtile[:], tp[:])
                    xt.append(xtile)
                # dk-pair-layout projections (q or k)
                for wsb, dst in qk_jobs:
                    for pair in range(NP):
                        ps = ps_big.tile([P, QR], F32, tag="psb")
                        for j in range(DCH):
                            nc.tensor.matmul(
                                ps[:], wsb[:, j, pair * P:(pair + 1) * P], xt[j][:],
                                start=(j == 0), stop=(j == DCH - 1))
                        half, col = qr // 2, (qr % 2) * QR
                        nc.any.tensor_copy(dst[pair][half][:, col:col + QR], ps[:])
                # v natural projection
                if with_v:
                    n_st = QR // P
                    for st in range(n_st):
                        pv = ps_mid.tile([P, HL * DK], F32, tag="psm", name="pv")
                        for j in range(DCH):
                            nc.tensor.matmul(
                                pv[:], xt[j][:, st * P:(st + 1) * P], wv_sb[:, j, :],
                                start=(j == 0), stop=(j == DCH - 1))
                        nc.any.tensor_copy(v_tiles[qr * n_st + st][:], pv[:])

            # ---- phase A: K+V from memory, then Q from tgt ----
            for qr in range(NQR):
                stream_quarter(mem_d, qr, [(wk_sb, kT)], with_v=True)
            for qr in range(NQR):
                stream_quarter(tgt_d, qr, [(wq_sb, qT)], with_v=False)

            # ---- phase B: attention ----
            for sqt in range(NSQT):
                half, col = sqt // 2, (sqt % 2) * SQT
                for pair in range(NP):
                    q1 = qT[pair][half][0:64, col:col + SQT]
                    q2 = qT[pair][half][64:P, col:col + SQT]
                    ps_o1 = ps_mid.tile([64, SQT], F32, tag="psm")
                    ps_o2 = ps_mid.tile([64, SQT], F32, tag="psm")
                    acc_e = acc_pool.tile([P, 2 * SQT], F32R, tag="acc")
                    acc_o = acc_pool.tile([P, 2 * SQT], F32R, tag="acc")
                    for skc in range(NSKC):
                        kh, kcol = skc // 8, (skc % 8) * P
                        ps_s = ps_big.tile([P, 2 * SQT], F32, tag="psb")
                        nc.tensor.matmul(
                            ps_s[:, 0:SQT], kT[pair][kh][0:64, kcol:kcol + P], q1,
                            start=True, stop=True, tile_position=(0, 0))
                        nc.tensor.matmul(
                            ps_s[:, SQT:2 * SQT], kT[pair][kh][64:P, kcol:kcol + P], q2,
                            start=True, stop=True, tile_position=(64, 0))
                        attn = attn_pool.tile([P, 2 * SQT], F32R, tag="attn")
                        nc.scalar.activation(attn[:], ps_s[:], AF.Exp, scale=SCALE)
                        if debug and sqt == 0 and pair == 0 and skc == 0:
                            nc.sync.dma_start(out=dbg_attn[:], in_=attn[:])
                        if skc == 0:
                            nc.vector.tensor_copy(acc_e[:], attn[:])
                        elif skc == 1:
                            nc.gpsimd.tensor_copy(acc_o[:], attn[:])
                        elif skc % 2 == 0:
                            nc.vector.tensor_add(acc_e[:], acc_e[:], attn[:])
                        else:
                            nc.gpsimd.tensor_add(acc_o[:], acc_o[:], attn[:])
                        h1, h2 = 2 * pair, 2 * pair + 1
                        nc.tensor.matmul(
                            ps_o1[:], v_tiles[skc][:, h1 * DK:(h1 + 1) * DK],
                            attn[:, 0:SQT],
                            start=(skc == 0), stop=(skc == NSKC - 1))
                        nc.tensor.matmul(
                            ps_o2[:], v_tiles[skc][:, h2 * DK:(h2 + 1) * DK],
                            attn[:, SQT:2 * SQT],
                            start=(skc == 0), stop=(skc == NSKC - 1))
                    # softmax denominators: partition-reduce the two accs
                    ps_m = ps_big.tile([1, 2 * SQT], F32, tag="psb")
                    nc.tensor.matmul(ps_m[0:1, 0:SQT], ones[:], acc_e[:, 0:SQT],
                                     start=True, stop=False)
                    nc.tensor.matmul(ps_m[0:1, 0:SQT], ones[:], acc_o[:, 0:SQT],
                                     start=False, stop=True)
                    nc.tensor.matmul(ps_m[0:1, SQT:2 * SQT], ones[:], acc_e[:, SQT:2 * SQT],
                                     start=True, stop=False)
                    nc.tensor.matmul(ps_m[0:1, SQT:2 * SQT], ones[:], acc_o[:, SQT:2 * SQT],
                                     start=False, stop=True)
                    rec = rec_pool.tile([1, 2 * SQT], F32R, tag="rec")
                    with nc.allow_low_precision(reason="f32r holds f32 bits"):
                        nc.vector.reciprocal(rec[0:1, 0:SQT], ps_m[0:1, 0:SQT])
                        nc.vector.reciprocal(rec[0:1, SQT:2 * SQT], ps_m[0:1, SQT:2 * SQT])
                    if debug and sqt == 0 and pair == 0:
                        nc.sync.dma_start(out=dbg_rec[:], in_=rec[:])
                    ps_b1 = ps_mid.tile([64, SQT], F32, tag="psm")
                    nc.tensor.matmul(ps_b1[:], sel[0:1, 0:64], rec[0:1, 0:SQT],
                                     start=True, stop=True)
                    ps_b2 = ps_mid.tile([64, SQT], F32, tag="psm")
                    nc.tensor.matmul(ps_b2[:], sel[0:1, 0:64], rec[0:1, SQT:2 * SQT],
                                     start=True, stop=True)
                    bc1 = rec_pool.tile([64, SQT], F32R, tag="bc")
                    bc2 = rec_pool.tile([64, SQT], F32R, tag="bc")
                    nc.vector.tensor_copy(bc1[:], ps_b1[:])
                    nc.vector.tensor_copy(bc2[:], ps_b2[:])
                    ot = outT[pair][sqt]
                    nc.vector.tensor_mul(ot[0:64, :], ps_o1[:], bc1[:])
                    tmp2 = rec_pool.tile([64, SQT], F32R, tag="tmp2")
                    nc.vector.tensor_mul(tmp2[:], ps_o2[:], bc2[:])
                    nc.sync.dma_start(out=ot[64:P, :], in_=tmp2[:])

            if debug:
                nc.sync.dma_start(out=dbg_qT[:], in_=qT[0][0][:])
                nc.sync.dma_start(out=dbg_kT[:], in_=kT[0][0][:])
                nc.sync.dma_start(out=dbg_v[:], in_=v_tiles[0][:])
                nc.sync.dma_start(out=dbg_outT[:], in_=outT[0][0][:])

            # ---- phase C: partial output projection ----
            for st in range(S // P):
                sqt = st // (SQT // P)
                c0 = (st % (SQT // P)) * P
                pso = [ps_mid.tile([P, D // 2], F32, tag="psm", name=f"pso_{ch}")
                       for ch in range(2)]
                for c in range(NP):
                    lhsT = outT[c][sqt][:, c0:c0 + P]
                    for ch in range(2):
                        nc.tensor.matmul(
                            pso[ch][:], lhsT, wo_sb[:, c, ch * (D // 2):(ch + 1) * (D // 2)],
                            start=(c == 0), stop=(c == NP - 1))
                stg = stage_pool.tile([P, D], F32R, tag="stage")
                nc.vector.tensor_copy(stg[:, 0:D // 2], pso[0][:])
                nc.vector.tensor_copy(stg[:, D // 2:D], pso[1][:])
                nc.sync.dma_start(out=out_d[st * P:(st + 1) * P, :], in_=stg[:])

    nc.compile()
    return nc


def _constants():
    sel = np.zeros((1, 2 * P), np.float32)
    sel[0, 0:64] = 1.0
    sel[0, P + 64:2 * P] = 1.0
    return {
        "ident": np.eye(P, dtype=np.float32),
        "ones": np.ones((P, 1), np.float32),
        "sel": sel,
    }


def kernel(tgt, memory, Wq, Wk, Wv, Wo, bo):
    tgt = np.asarray(tgt, dtype=np.float32)
    memory = np.asarray(memory, dtype=np.float32)
    Wq = np.asarray(Wq, dtype=np.float32)
    Wk = np.asarray(Wk, dtype=np.float32)
    Wv = np.asarray(Wv, dtype=np.float32)
    Wo = np.asarray(Wo, dtype=np.float32)
    bo = np.asarray(bo, dtype=np.float32)

    if "nc" not in _CACHE:
        _CACHE["nc"] = build_nc()
    nc = _CACHE["nc"]

    consts = _constants()
    in_maps = []
    for c in range(8):
        b, g = c // 4, c % 4
        hs = slice(g * HL, (g + 1) * HL)
        in_maps.append({
            "tgt": np.ascontiguousarray(tgt[b]),
            "mem": np.ascontiguousarray(memory[b]),
            # [HL, D, DK] -> [D, HL*DK] head-major columns
            "wq": np.ascontiguousarray(Wq[hs].transpose(1, 0, 2).reshape(D, HL * DK)),
            "wk": np.ascontiguousarray(Wk[hs].transpose(1, 0, 2).reshape(D, HL * DK)),
            "wv": np.ascontiguousarray(Wv[hs].transpose(1, 0, 2).reshape(D, HL * DK)),
            "wo": np.ascontiguousarray(Wo[g * HL * DK:(g + 1) * HL * DK, :]),
            **consts,
        })

    res = run_bass_kernel_spmd(nc, in_maps, list(range(8)))
    out = np.zeros((B, S, D), np.float32)
    for c in range(8):
        out[c // 4] += res.results[c]["out"]
    out += bo[None, None, :]
    return out


if __name__ == "__main__":
    pass


# revision 16
# speedup vs baseline: 1.1179x; 1.1179x over previous
"""Trainium2 Bass kernel for per-head-projection MHA (NaiveMHA).

Problem: B=2, S=2048, D=1024, H=16, DK=64.
  q = einsum('bqd,hdk->bhqk', tgt, Wq); k,v likewise from memory
  attn = softmax(q k^T / sqrt(DK)); out = (attn v concat-heads) @ Wo + bo

Sharding over 8 cores: data-parallel on batch (2) x tensor-parallel on
heads (4 local heads per core).  Each core computes its 4 heads end to
end plus the partial output projection through its slice of Wo's rows;
the host sums the 4 partials per batch and adds bo.

Per-core dataflow (all matmuls fp32r = full-rate 4-byte):
  - stream tgt/memory through PE transposes (identity matmul) to get
    d-major activation chunks
  - qT/kT computed as [dk-pair(128), S] with two heads stacked on the
    partition dim; v computed in natural [S, 4*DK] layout
  - attention in scoresT orientation [sk, sq]: paired K=64 matmuls via
    tile_position row groups, one Exp instruction per head-pair chunk,
    A*V accumulated over sk in PSUM via tile_position col groups,
    softmax row sums via ones-matmul partition reduction, reciprocal,
    K=2 selector-matmul broadcast, normalization applied to the small
    [dk, sq] A*V output (softmax denominator deferred past A*V).
"""

import numpy as np
import ml_dtypes

_bf16 = np.dtype(ml_dtypes.bfloat16)

import concourse.mybir as mybir
import concourse.tile as tile
from concourse import bacc
from concourse.bass_utils import run_bass_kernel_spmd

F32R = mybir.dt.float32r
F32 = mybir.dt.float32
BF16 = mybir.dt.bfloat16
AF = mybir.ActivationFunctionType

B, S, D, H = 2, 2048, 1024, 16
DK = D // H
HL = 4  # local heads per core
NP = 2  # local head pairs
P = 128
DCH = D // P  # 8 d-chunks
QR = 512  # row quarter for streaming transposes
NQR = S // QR  # 4
SQT = 512  # sq tile in attention
NSQT = S // SQT  # 4
NSKC = S // P  # 16 sk chunks
SCALE = 1.0 / float(np.sqrt(DK))

_CACHE = {}


def build_nc(debug=False):
    nc = bacc.Bacc("TRN2", target_bir_lowering=False, debug=False, num_devices=8)

    tgt_d = nc.declare_dram_parameter("tgt", [S, D], F32R, isOutput=False)
    mem_d = nc.declare_dram_parameter("mem", [S, D], F32R, isOutput=False)
    wq_d = nc.declare_dram_parameter("wq", [D, HL * DK], F32R, isOutput=False)
    wk_d = nc.declare_dram_parameter("wk", [D, HL * DK], F32R, isOutput=False)
    wv_d = nc.declare_dram_parameter("wv", [D, HL * DK], F32R, isOutput=False)
    wo_d = nc.declare_dram_parameter("wo", [HL * DK, D], F32R, isOutput=False)
    ident_d = nc.declare_dram_parameter("ident", [P, P], F32R, isOutput=False)
    ones4_d = nc.declare_dram_parameter("ones4", [P, HL], BF16, isOutput=False)
    sel_d = nc.declare_dram_parameter("sel", [1, 2 * P], F32R, isOutput=False)
    out_d = nc.declare_dram_parameter("out", [S, D], F32R, isOutput=True)
    if debug:
        dbg_qT = nc.declare_dram_parameter("dbg_qT", [P, S // 2], F32R, isOutput=True)
        dbg_kT = nc.declare_dram_parameter("dbg_kT", [P, S // 2], F32R, isOutput=True)
        dbg_v = nc.declare_dram_parameter("dbg_v", [P, HL * (DK + 2)], BF16, isOutput=True)
        dbg_rec = nc.declare_dram_parameter("dbg_rec", [1, 2 * SQT], F32R, isOutput=True)
        dbg_outT = nc.declare_dram_parameter("dbg_outT", [P, SQT], F32R, isOutput=True)
        dbg_attn = nc.declare_dram_parameter("dbg_attn", [P, 2 * SQT], BF16, isOutput=True)

    with tile.TileContext(nc) as tc:
        with (
            tc.tile_pool(name="const", bufs=1) as const,
            tc.tile_pool(name="w", bufs=1) as wpool,
            tc.tile_pool(name="nat", bufs=6) as nat_pool,
            tc.tile_pool(name="xt", bufs=10) as xt_pool,
            tc.tile_pool(name="qk", bufs=8) as qk_pool,
            tc.tile_pool(name="vp", bufs=16) as v_pool,
            tc.tile_pool(name="attn", bufs=4) as attn_pool,
            tc.tile_pool(name="rec", bufs=2) as rec_pool,
            tc.tile_pool(name="outt", bufs=8) as outt_pool,
            tc.tile_pool(name="stage", bufs=3) as stage_pool,
            tc.tile_pool(name="psb", bufs=2, space="PSUM") as ps_big,
            tc.tile_pool(name="psm", bufs=3, space="PSUM") as ps_mid,
        ):
            # constants
            ident = const.tile([P, P], F32R, tag="ident")
            nc.sync.dma_start(out=ident[:], in_=ident_d[:])
            sel = const.tile([1, 2 * P], F32R, tag="sel")
            nc.sync.dma_start(out=sel[:], in_=sel_d[:])
            # weights
            wq_sb = wpool.tile([P, DCH, HL * DK], F32R, tag="wq")
            nc.sync.dma_start(out=wq_sb[:], in_=wq_d.ap().rearrange("(c p) n -> p c n", p=P))
            wk_sb = wpool.tile([P, DCH, HL * DK], F32R, tag="wk")
            nc.sync.dma_start(out=wk_sb[:], in_=wk_d.ap().rearrange("(c p) n -> p c n", p=P))
            wv_sb = wpool.tile([P, DCH, HL * DK], F32R, tag="wv")
            nc.sync.dma_start(out=wv_sb[:], in_=wv_d.ap().rearrange("(c p) n -> p c n", p=P))
            wo_sb = wpool.tile([P, NP, D], F32R, tag="wo")
            nc.sync.dma_start(out=wo_sb[:], in_=wo_d.ap().rearrange("(c p) n -> p c n", p=P))

            # persistent intermediate tiles
            kT = [[qk_pool.tile([P, S // 2], F32R, tag="qk", name=f"kT_{p}_{h}")
                   for h in range(2)] for p in range(NP)]
            qT = [[qk_pool.tile([P, S // 2], F32R, tag="qk", name=f"qT_{p}_{h}")
                   for h in range(2)] for p in range(NP)]
            v_tiles = [v_pool.tile([P, HL * (DK + 2)], BF16, tag="v", name=f"v_{i}")
                       for i in range(S // P)]
            outT = [[outt_pool.tile([P, SQT], F32R, tag="outt", name=f"outT_{p}_{t}")
                     for t in range(NSQT)] for p in range(NP)]

            def stream_quarter(src_dram, qr, qk_jobs, with_v):
                """Load rows [qr*QR, (qr+1)*QR) of src, transpose to d-major
                chunks, then run the requested projections at this quarter."""
                nat = []
                for t in range(QR // P):
                    ntile = nat_pool.tile([P, D], F32R, tag="nat")
                    r0 = qr * QR + t * P
                    nc.sync.dma_start(out=ntile[:], in_=src_dram[r0:r0 + P, :])
                    nat.append(ntile)
                xt = []
                for j in range(DCH):
                    xtile = xt_pool.tile([P, QR], F32R, tag="xt")
                    tp = ps_mid.tile([P, QR], F32R, tag="psm")
                    for t in range(QR // P):
                        nc.tensor.transpose(
                            tp[:, t * P:(t + 1) * P], nat[t][:, j * P:(j + 1) * P], ident[:])
                    nc.any.tensor_copy(xtile[:], tp[:])
                    xt.append(xtile)
                # dk-pair-layout projections (q or k)
                for wsb, dst in qk_jobs:
                    for pair in range(NP):
                        ps = ps_big.tile([P, QR], F32, tag="psb")
                        for j in range(DCH):
                            nc.tensor.matmul(
                                ps[:], wsb[:, j, pair * P:(pair + 1) * P], xt[j][:],
                                start=(j == 0), stop=(j == DCH - 1))
                        half, col = qr // 2, (qr % 2) * QR
                        nc.any.tensor_copy(dst[pair][half][:, col:col + QR], ps[:])
                # v natural projection
                if with_v:
                    n_st = QR // P
                    for st in range(n_st):
                        pv = ps_mid.tile([P, HL * DK], F32, tag="psm", name="pv")
                        for j in range(DCH):
                            nc.tensor.matmul(
                                pv[:], xt[j][:, st * P:(st + 1) * P], wv_sb[:, j, :],
                                start=(j == 0), stop=(j == DCH - 1))
                        vt = v_tiles[qr * n_st + st].rearrange("p (h x) -> p h x", x=DK + 2)
                        nc.any.tensor_copy(
                            vt[:, :, 0:DK],
                            pv[:].rearrange("p (h k) -> p h k", k=DK))
                        nc.sync.dma_start(out=vt[:, :, DK], in_=ones4_d[:])

            # ---- phase A: K+V from memory, then Q from tgt ----
            for qr in range(NQR):
                stream_quarter(mem_d, qr, [(wk_sb, kT)], with_v=True)
            for qr in range(NQR):
                stream_quarter(tgt_d, qr, [(wq_sb, qT)], with_v=False)

            # ---- phase B: attention (+ interleaved output projection) ----
            def out_proj(st):
                sqt = st // (SQT // P)
                c0 = (st % (SQT // P)) * P
                pso = [ps_mid.tile([P, D // 2], F32, tag="psm", name=f"pso_{ch}")
                       for ch in range(2)]
                for c in range(NP):
                    lhsT = outT[c][sqt][:, c0:c0 + P]
                    for ch in range(2):
                        nc.tensor.matmul(
                            pso[ch][:], lhsT, wo_sb[:, c, ch * (D // 2):(ch + 1) * (D // 2)],
                            start=(c == 0), stop=(c == NP - 1))
                stg = stage_pool.tile([P, D], F32R, tag="stage")
                nc.vector.tensor_copy(stg[:, 0:D // 2], pso[0][:])
                nc.vector.tensor_copy(stg[:, D // 2:D], pso[1][:])
                nc.sync.dma_start(out=out_d[st * P:(st + 1) * P, :], in_=stg[:])

            for sqt in range(NSQT):
                half, col = sqt // 2, (sqt % 2) * SQT
                for pair in range(NP):
                    q1 = qT[pair][half][0:64, col:col + SQT]
                    q2 = qT[pair][half][64:P, col:col + SQT]
                    ps_o1 = ps_mid.tile([65, SQT], F32, tag="psm")
                    ps_o2 = ps_mid.tile([65, SQT], F32, tag="psm")
                    for skc in range(NSKC):
                        kh, kcol = skc // 8, (skc % 8) * P
                        ps_s = ps_big.tile([P, 2 * SQT], F32, tag="psb")
                        nc.tensor.matmul(
                            ps_s[:, 0:SQT], kT[pair][kh][0:64, kcol:kcol + P], q1,
                            start=True, stop=True, tile_position=(0, 0))
                        nc.tensor.matmul(
                            ps_s[:, SQT:2 * SQT], kT[pair][kh][64:P, kcol:kcol + P], q2,
                            start=True, stop=True, tile_position=(64, 0))
                        attn = attn_pool.tile([P, 2 * SQT], BF16, tag="attn")
                        nc.scalar.activation(attn[:], ps_s[:], AF.Exp, scale=SCALE)
                        if debug and sqt == 0 and pair == 0 and skc == 0:
                            nc.sync.dma_start(out=dbg_attn[:], in_=attn[:])
                        h1, h2 = 2 * pair, 2 * pair + 1
                        nc.tensor.matmul(
                            ps_o1[:], v_tiles[skc][:, h1 * (DK + 2):h1 * (DK + 2) + DK + 1],
                            attn[:, 0:SQT],
                            start=(skc == 0), stop=(skc == NSKC - 1))
                        nc.tensor.matmul(
                            ps_o2[:], v_tiles[skc][:, h2 * (DK + 2):h2 * (DK + 2) + DK + 1],
                            attn[:, SQT:2 * SQT],
                            start=(skc == 0), stop=(skc == NSKC - 1))
                    # softmax denominators sit in ps_o[64:65] (ones column of v)
                    rec = rec_pool.tile([1, 2 * SQT], F32R, tag="rec")
                    for hh, ps_oh in ((0, ps_o1), (1, ps_o2)):
                        s_sb = rec_pool.tile([1, SQT], F32R, tag="s_sb")
                        nc.vector.tensor_copy(s_sb[:], ps_oh[64:65, :])
                        s128 = rec_pool.tile([P, SQT // P, 2], F32R, tag="s128")
                        nc.sync.dma_start(
                            out=s128[:, :, 0],
                            in_=s_sb[0:1, :].rearrange("o (p f) -> o p f", p=P))
                        with nc.allow_low_precision(reason="f32r holds f32 bits"):
                            nc.vector.reciprocal(s128[:, :, 1], s128[:, :, 0])
                        nc.sync.dma_start(
                            out=rec[0:1, hh * SQT:(hh + 1) * SQT].rearrange(
                                "o (p f) -> o p f", p=P),
                            in_=s128[:, :, 1])
                    if debug and sqt == 0 and pair == 0:
                        nc.sync.dma_start(out=dbg_rec[:], in_=rec[:])
                    ps_b1 = ps_mid.tile([64, SQT], F32, tag="psm")
                    nc.tensor.matmul(ps_b1[:], sel[0:1, 0:64], rec[0:1, 0:SQT],
                                     start=True, stop=True)
                    ps_b2 = ps_mid.tile([64, SQT], F32, tag="psm")
                    nc.tensor.matmul(ps_b2[:], sel[0:1, 0:64], rec[0:1, SQT:2 * SQT],
                                     start=True, stop=True)
                    bc1 = rec_pool.tile([64, SQT], F32R, tag="bc")
                    bc2 = rec_pool.tile([64, SQT], F32R, tag="bc")
                    nc.any.tensor_copy(bc1[:], ps_b1[:])
                    nc.any.tensor_copy(bc2[:], ps_b2[:])
                    ot = outT[pair][sqt]
                    nc.vector.tensor_mul(ot[0:64, :], ps_o1[0:64, :], bc1[:])
                    tmp2 = rec_pool.tile([64, SQT], F32R, tag="tmp2")
                    nc.vector.tensor_mul(tmp2[:], ps_o2[0:64, :], bc2[:])
                    nc.sync.dma_start(out=ot[64:P, :], in_=tmp2[:])
                for st in range(sqt * (SQT // P), (sqt + 1) * (SQT // P)):
                    out_proj(st)

            if debug:
                nc.sync.dma_start(out=dbg_qT[:], in_=qT[0][0][:])
                nc.sync.dma_start(out=dbg_kT[:], in_=kT[0][0][:])
                nc.sync.dma_start(out=dbg_v[:], in_=v_tiles[0][:])
                nc.sync.dma_start(out=dbg_outT[:], in_=outT[0][0][:])

    nc.compile()
    return nc


def _constants():
    sel = np.zeros((1, 2 * P), np.float32)
    sel[0, 0:64] = 1.0
    sel[0, P + 64:2 * P] = 1.0
    return {
        "ident": np.eye(P, dtype=np.float32),
        "ones4": np.ones((P, HL), _bf16),
        "sel": sel,
    }


def kernel(tgt, memory, Wq, Wk, Wv, Wo, bo):
    tgt = np.asarray(tgt, dtype=np.float32)
    memory = np.asarray(memory, dtype=np.float32)
    Wq = np.asarray(Wq, dtype=np.float32)
    Wk = np.asarray(Wk, dtype=np.float32)
    Wv = np.asarray(Wv, dtype=np.float32)
    Wo = np.asarray(Wo, dtype=np.float32)
    bo = np.asarray(bo, dtype=np.float32)

    if "nc" not in _CACHE:
        _CACHE["nc"] = build_nc()
    nc = _CACHE["nc"]

    consts = _constants()
    in_maps = []
    for c in range(8):
        b, g = c // 4, c % 4
        hs = slice(g * HL, (g + 1) * HL)
        in_maps.append({
            "tgt": np.ascontiguousarray(tgt[b]),
            "mem": np.ascontiguousarray(memory[b]),
            # [HL, D, DK] -> [D, HL*DK] head-major columns
            "wq": np.ascontiguousarray(Wq[hs].transpose(1, 0, 2).reshape(D, HL * DK)),
            "wk": np.ascontiguousarray(Wk[hs].transpose(1, 0, 2).reshape(D, HL * DK)),
            "wv": np.ascontiguousarray(Wv[hs].transpose(1, 0, 2).reshape(D, HL * DK)),
            "wo": np.ascontiguousarray(Wo[g * HL * DK:(g + 1) * HL * DK, :]),
            **consts,
        })

    res = run_bass_kernel_spmd(nc, in_maps, list(range(8)))
    out = np.zeros((B, S, D), np.float32)
    for c in range(8):
        out[c // 4] += res.results[c]["out"]
    out += bo[None, None, :]
    return out


if __name__ == "__main__":
    pass


# revision 17
# speedup vs baseline: 1.1718x; 1.0482x over previous
"""Trainium2 Bass kernel for per-head-projection MHA (NaiveMHA).

Problem: B=2, S=2048, D=1024, H=16, DK=64.
  q = einsum('bqd,hdk->bhqk', tgt, Wq); k,v likewise from memory
  attn = softmax(q k^T / sqrt(DK)); out = (attn v concat-heads) @ Wo + bo

Sharding over 8 cores: data-parallel on batch (2) x tensor-parallel on
heads (4 local heads per core).  Each core computes its 4 heads end to
end plus the partial output projection through its slice of Wo's rows;
the host sums the 4 partials per batch and adds bo.

Per-core dataflow (all matmuls fp32r = full-rate 4-byte):
  - stream tgt/memory through PE transposes (identity matmul) to get
    d-major activation chunks
  - qT/kT computed as [dk-pair(128), S] with two heads stacked on the
    partition dim; v computed in natural [S, 4*DK] layout
  - attention in scoresT orientation [sk, sq]: paired K=64 matmuls via
    tile_position row groups, one Exp instruction per head-pair chunk,
    A*V accumulated over sk in PSUM via tile_position col groups,
    softmax row sums via ones-matmul partition reduction, reciprocal,
    K=2 selector-matmul broadcast, normalization applied to the small
    [dk, sq] A*V output (softmax denominator deferred past A*V).
"""

import numpy as np
import ml_dtypes

_bf16 = np.dtype(ml_dtypes.bfloat16)

import concourse.mybir as mybir
import concourse.tile as tile
from concourse import bacc
from concourse.bass_utils import run_bass_kernel_spmd

F32R = mybir.dt.float32r
F32 = mybir.dt.float32
BF16 = mybir.dt.bfloat16
AF = mybir.ActivationFunctionType

B, S, D, H = 2, 2048, 1024, 16
DK = D // H
HL = 4  # local heads per core
NP = 2  # local head pairs
P = 128
DCH = D // P  # 8 d-chunks
QR = 512  # row quarter for streaming transposes
NQR = S // QR  # 4
SQT = 512  # sq tile in attention
NSQT = S // SQT  # 4
NSKC = S // P  # 16 sk chunks
SCALE = 1.0 / float(np.sqrt(DK))

_CACHE = {}


def build_nc(debug=False):
    nc = bacc.Bacc("TRN2", target_bir_lowering=False, debug=False, num_devices=8)

    tgtT_d = nc.declare_dram_parameter("tgtT", [D, S], F32R, isOutput=False)
    memT_d = nc.declare_dram_parameter("memT", [D, S], F32R, isOutput=False)
    wq_d = nc.declare_dram_parameter("wq", [D, HL * DK], F32R, isOutput=False)
    wk_d = nc.declare_dram_parameter("wk", [D, HL * DK], F32R, isOutput=False)
    wv_d = nc.declare_dram_parameter("wv", [D, HL * DK], F32R, isOutput=False)
    wo_d = nc.declare_dram_parameter("wo", [HL * DK, D], F32R, isOutput=False)
    ones4_d = nc.declare_dram_parameter("ones4", [P, HL], BF16, isOutput=False)
    sel_d = nc.declare_dram_parameter("sel", [1, 2 * P], F32R, isOutput=False)
    out_d = nc.declare_dram_parameter("out", [S, D], F32R, isOutput=True)
    if debug:
        dbg_qT = nc.declare_dram_parameter("dbg_qT", [P, S // 2], F32R, isOutput=True)
        dbg_kT = nc.declare_dram_parameter("dbg_kT", [P, S // 2], F32R, isOutput=True)
        dbg_v = nc.declare_dram_parameter("dbg_v", [P, HL * (DK + 2)], BF16, isOutput=True)
        dbg_rec = nc.declare_dram_parameter("dbg_rec", [1, 2 * SQT], F32R, isOutput=True)
        dbg_outT = nc.declare_dram_parameter("dbg_outT", [P, SQT], F32R, isOutput=True)
        dbg_attn = nc.declare_dram_parameter("dbg_attn", [P, 2 * SQT], BF16, isOutput=True)

    with tile.TileContext(nc) as tc:
        with (
            tc.tile_pool(name="const", bufs=1) as const,
            tc.tile_pool(name="w", bufs=1) as wpool,
            tc.tile_pool(name="xt", bufs=10) as xt_pool,
            tc.tile_pool(name="qk", bufs=8) as qk_pool,
            tc.tile_pool(name="vp", bufs=16) as v_pool,
            tc.tile_pool(name="attn", bufs=4) as attn_pool,
            tc.tile_pool(name="rec", bufs=2) as rec_pool,
            tc.tile_pool(name="outt", bufs=8) as outt_pool,
            tc.tile_pool(name="stage", bufs=3) as stage_pool,
            tc.tile_pool(name="psb", bufs=2, space="PSUM") as ps_big,
            tc.tile_pool(name="psm", bufs=3, space="PSUM") as ps_mid,
        ):
            # constants
            sel = const.tile([1, 2 * P], F32R, tag="sel")
            nc.sync.dma_start(out=sel[:], in_=sel_d[:])
            # weights
            wq_sb = wpool.tile([P, DCH, HL * DK], F32R, tag="wq")
            nc.sync.dma_start(out=wq_sb[:], in_=wq_d.ap().rearrange("(c p) n -> p c n", p=P))
            wk_sb = wpool.tile([P, DCH, HL * DK], F32R, tag="wk")
            nc.sync.dma_start(out=wk_sb[:], in_=wk_d.ap().rearrange("(c p) n -> p c n", p=P))
            wv_sb = wpool.tile([P, DCH, HL * DK], F32R, tag="wv")
            nc.sync.dma_start(out=wv_sb[:], in_=wv_d.ap().rearrange("(c p) n -> p c n", p=P))
            wo_sb = wpool.tile([P, NP, D], F32R, tag="wo")
            nc.sync.dma_start(out=wo_sb[:], in_=wo_d.ap().rearrange("(c p) n -> p c n", p=P))

            # persistent intermediate tiles
            kT = [[qk_pool.tile([P, S // 2], F32R, tag="qk", name=f"kT_{p}_{h}")
                   for h in range(2)] for p in range(NP)]
            qT = [[qk_pool.tile([P, S // 2], F32R, tag="qk", name=f"qT_{p}_{h}")
                   for h in range(2)] for p in range(NP)]
            v_tiles = [v_pool.tile([P, HL * (DK + 2)], BF16, tag="v", name=f"v_{i}")
                       for i in range(S // P)]
            outT = [[outt_pool.tile([P, SQT], F32R, tag="outt", name=f"outT_{p}_{t}")
                     for t in range(NSQT)] for p in range(NP)]

            def stream_quarter(srcT_dram, qr, qk_jobs, with_v):
                """Load the d-major chunks of seq rows [qr*QR, (qr+1)*QR)."""
                xt = []
                for j in range(DCH):
                    xtile = xt_pool.tile([P, QR], F32R, tag="xt")
                    nc.sync.dma_start(
                        out=xtile[:],
                        in_=srcT_dram[j * P:(j + 1) * P, qr * QR:(qr + 1) * QR])
                    xt.append(xtile)
                # dk-pair-layout projections (q or k)
                for wsb, dst in qk_jobs:
                    for pair in range(NP):
                        ps = ps_big.tile([P, QR], F32, tag="psb")
                        for j in range(DCH):
                            nc.tensor.matmul(
                                ps[:], wsb[:, j, pair * P:(pair + 1) * P], xt[j][:],
                                start=(j == 0), stop=(j == DCH - 1))
                        half, col = qr // 2, (qr % 2) * QR
                        nc.any.tensor_copy(dst[pair][half][:, col:col + QR], ps[:])
                # v natural projection
                if with_v:
                    n_st = QR // P
                    for st in range(n_st):
                        pv = ps_mid.tile([P, HL * DK], F32, tag="psm", name="pv")
                        for j in range(DCH):
                            nc.tensor.matmul(
                                pv[:], xt[j][:, st * P:(st + 1) * P], wv_sb[:, j, :],
                                start=(j == 0), stop=(j == DCH - 1))
                        vt = v_tiles[qr * n_st + st].rearrange("p (h x) -> p h x", x=DK + 2)
                        nc.any.tensor_copy(
                            vt[:, :, 0:DK],
                            pv[:].rearrange("p (h k) -> p h k", k=DK))
                        nc.sync.dma_start(out=vt[:, :, DK], in_=ones4_d[:])

            # ---- phase A: K+V from memory, then Q from tgt ----
            for qr in range(NQR):
                stream_quarter(memT_d, qr, [(wk_sb, kT)], with_v=True)
            for qr in range(NQR):
                stream_quarter(tgtT_d, qr, [(wq_sb, qT)], with_v=False)

            # ---- phase B: attention (+ interleaved output projection) ----
            def out_proj(st):
                sqt = st // (SQT // P)
                c0 = (st % (SQT // P)) * P
                pso = [ps_mid.tile([P, D // 2], F32, tag="psm", name=f"pso_{ch}")
                       for ch in range(2)]
                for c in range(NP):
                    lhsT = outT[c][sqt][:, c0:c0 + P]
                    for ch in range(2):
                        nc.tensor.matmul(
                            pso[ch][:], lhsT, wo_sb[:, c, ch * (D // 2):(ch + 1) * (D // 2)],
                            start=(c == 0), stop=(c == NP - 1))
                stg = stage_pool.tile([P, D], F32R, tag="stage")
                nc.vector.tensor_copy(stg[:, 0:D // 2], pso[0][:])
                nc.vector.tensor_copy(stg[:, D // 2:D], pso[1][:])
                nc.sync.dma_start(out=out_d[st * P:(st + 1) * P, :], in_=stg[:])

            for sqt in range(NSQT):
                half, col = sqt // 2, (sqt % 2) * SQT
                for pair in range(NP):
                    q1 = qT[pair][half][0:64, col:col + SQT]
                    q2 = qT[pair][half][64:P, col:col + SQT]
                    ps_o1 = ps_mid.tile([65, SQT], F32, tag="psm")
                    ps_o2 = ps_mid.tile([65, SQT], F32, tag="psm")
                    for skc in range(NSKC):
                        kh, kcol = skc // 8, (skc % 8) * P
                        ps_s = ps_big.tile([P, 2 * SQT], F32, tag="psb")
                        nc.tensor.matmul(
                            ps_s[:, 0:SQT], kT[pair][kh][0:64, kcol:kcol + P], q1,
                            start=True, stop=True, tile_position=(0, 0))
                        nc.tensor.matmul(
                            ps_s[:, SQT:2 * SQT], kT[pair][kh][64:P, kcol:kcol + P], q2,
                            start=True, stop=True, tile_position=(64, 0))
                        attn = attn_pool.tile([P, 2 * SQT], BF16, tag="attn")
                        nc.scalar.activation(attn[:], ps_s[:], AF.Exp, scale=SCALE)
                        if debug and sqt == 0 and pair == 0 and skc == 0:
                            nc.sync.dma_start(out=dbg_attn[:], in_=attn[:])
                        h1, h2 = 2 * pair, 2 * pair + 1
                        nc.tensor.matmul(
                            ps_o1[:], v_tiles[skc][:, h1 * (DK + 2):h1 * (DK + 2) + DK + 1],
                            attn[:, 0:SQT],
                            start=(skc == 0), stop=(skc == NSKC - 1))
                        nc.tensor.matmul(
                            ps_o2[:], v_tiles[skc][:, h2 * (DK + 2):h2 * (DK + 2) + DK + 1],
                            attn[:, SQT:2 * SQT],
                            start=(skc == 0), stop=(skc == NSKC - 1))
                    # softmax denominators sit in ps_o[64:65] (ones column of v)
                    rec = rec_pool.tile([1, 2 * SQT], F32R, tag="rec")
                    for hh, ps_oh in ((0, ps_o1), (1, ps_o2)):
                        s_sb = rec_pool.tile([1, SQT], F32R, tag="s_sb")
                        nc.vector.tensor_copy(s_sb[:], ps_oh[64:65, :])
                        s128 = rec_pool.tile([P, SQT // P, 2], F32R, tag="s128")
                        nc.sync.dma_start(
                            out=s128[:, :, 0],
                            in_=s_sb[0:1, :].rearrange("o (p f) -> o p f", p=P))
                        with nc.allow_low_precision(reason="f32r holds f32 bits"):
                            nc.vector.reciprocal(s128[:, :, 1], s128[:, :, 0])
                        nc.sync.dma_start(
                            out=rec[0:1, hh * SQT:(hh + 1) * SQT].rearrange(
                                "o (p f) -> o p f", p=P),
                            in_=s128[:, :, 1])
                    if debug and sqt == 0 and pair == 0:
                        nc.sync.dma_start(out=dbg_rec[:], in_=rec[:])
                    bc1 = rec_pool.tile([64, SQT], F32R, tag="bc")
                    bc2 = rec_pool.tile([64, SQT], F32R, tag="bc")
                    nc.gpsimd.partition_broadcast(bc1[:], rec[0:1, 0:SQT])
                    nc.gpsimd.partition_broadcast(bc2[:], rec[0:1, SQT:2 * SQT])
                    ot = outT[pair][sqt]
                    nc.vector.tensor_mul(ot[0:64, :], ps_o1[0:64, :], bc1[:])
                    tmp2 = rec_pool.tile([64, SQT], F32R, tag="tmp2")
                    nc.vector.tensor_mul(tmp2[:], ps_o2[0:64, :], bc2[:])
                    nc.sync.dma_start(out=ot[64:P, :], in_=tmp2[:])
                for st in range(sqt * (SQT // P), (sqt + 1) * (SQT // P)):
                    out_proj(st)

            if debug:
                nc.sync.dma_start(out=dbg_qT[:], in_=qT[0][0][:])
                nc.sync.dma_start(out=dbg_kT[:], in_=kT[0][0][:])
                nc.sync.dma_start(out=dbg_v[:], in_=v_tiles[0][:])
                nc.sync.dma_start(out=dbg_outT[:], in_=outT[0][0][:])

    nc.compile()
    return nc


def _constants():
    sel = np.zeros((1, 2 * P), np.float32)
    sel[0, 0:64] = 1.0
    sel[0, P + 64:2 * P] = 1.0
    return {
        "ones4": np.ones((P, HL), _bf16),
        "sel": sel,
    }


def kernel(tgt, memory, Wq, Wk, Wv, Wo, bo):
    tgt = np.asarray(tgt, dtype=np.float32)
    memory = np.asarray(memory, dtype=np.float32)
    Wq = np.asarray(Wq, dtype=np.float32)
    Wk = np.asarray(Wk, dtype=np.float32)
    Wv = np.asarray(Wv, dtype=np.float32)
    Wo = np.asarray(Wo, dtype=np.float32)
    bo = np.asarray(bo, dtype=np.float32)

    if "nc" not in _CACHE:
        _CACHE["nc"] = build_nc()
    nc = _CACHE["nc"]

    consts = _constants()
    in_maps = []
    for c in range(8):
        b, g = c // 4, c % 4
        hs = slice(g * HL, (g + 1) * HL)
        in_maps.append({
            "tgtT": np.ascontiguousarray(tgt[b].T),
            "memT": np.ascontiguousarray(memory[b].T),
            # [HL, D, DK] -> [D, HL*DK] head-major columns
            "wq": np.ascontiguousarray(Wq[hs].transpose(1, 0, 2).reshape(D, HL * DK)),
            "wk": np.ascontiguousarray(Wk[hs].transpose(1, 0, 2).reshape(D, HL * DK)),
            "wv": np.ascontiguousarray(Wv[hs].transpose(1, 0, 2).reshape(D, HL * DK)),
            "wo": np.ascontiguousarray(Wo[g * HL * DK:(g + 1) * HL * DK, :]),
            **consts,
        })

    res = run_bass_kernel_spmd(nc, in_maps, list(range(8)))
    out = np.zeros((B, S, D), np.float32)
    for c in range(8):
        out[c // 4] += res.results[c]["out"]
    out += bo[None, None, :]
    return out


if __name__ == "__main__":
    pass
